# revision 31
# baseline (speedup 1.0000x reference)
"""TGN message-passing kernel for 8 Trainium2 NeuronCores.

Strategy:
  - Sort edges by destination; shard edges across cores by destination node
    range (6272 nodes/core). No collectives needed.
  - Per 128-node window: segment-sum via one-hot matmuls accumulated in PSUM
    (src-memory chunk + [timeenc|edgefeat] chunk + count row), then the
    dst-memory chunk analytically = count[n] * dst_mem[n] (no gather).
  - GRU fused per window: gi/gh matmuls accumulate in PSUM, gates on ACT/DVE,
    output written transposed; host re-transposes and trims.
"""
import os
import sys
sys.path.insert(0, '/opt/trn_rl_repo')
import numpy as np
from concourse import bass, bacc, tile, mybir
from concourse import bass_utils
from concourse.tile import add_dep_helper

F32 = mybir.dt.float32
I32 = mybir.dt.int32
I16 = mybir.dt.int16

NN = 50000      # nodes per side (users == items == 50000)
EDG = 400000
S = 128         # memory dim
TD = 64         # time-encode dim
ED = 64         # edge-feat dim
NCORES = 8
NW = 49         # windows per core per graph
NPC = NW * 128  # 6272 nodes per core
NPAD = NPC * NCORES  # 50176
NZT = 6         # tiles per src zone (zone A: src < HALF, zone B: src >= HALF)
TW = 2 * NZT    # 12 edge tiles (of 128) per window
NZS = NZT * 128  # 768 slots per zone
HALF = 25088    # src-table zone split (int16 gather index limit)
G = 7           # windows per e/t/d batch group
NG = NW // G    # 7 groups
GT = G * TW     # 84 tiles per group

STATIC_COUNTS = os.environ.get("STATIC_COUNTS", "0") == "1"

_CACHE = {}


def _dep_on_prev_readers(inst, state, tag, bufs=2):
    """Sync the first writer of a reused PSUM slot on the readers of the
    tile that previously occupied it (Tile's WAR enforcement misses this)."""
    hist = state.setdefault(tag, [])
    if len(hist) >= bufs:
        for rd in hist[-bufs]:
            add_dep_helper(inst.ins, rd.ins, sync=True,
                           reason=f"WAR: {tag} slot reuse")


def _emit_graph(nc, tc, pools, cst, tensors, war_state, regs):
    """Emit one graph's 49 windows."""
    (gpool, wpool, ps_seg, ps_bc, ps_a, ps_b) = pools
    (wih_t, whh_t, bias4, iota_t, freqbc_t, halfpi_t, ones_col, ones_row) = cst
    (src_tab, hT_d, idx_d, cnt_d, e_d, t_d, d_d, out_d) = tensors
    regA, regB = regs

    cnt_t = gpool.tile([1, 2 * NW], I32, tag="cnt")
    nc.sync.dma_start(cnt_t[:], cnt_d[:])

    for grp in range(NG):
        chunk = gpool.tile([128, GT * 128], F32, tag="chunk")
        ch3 = chunk[:].rearrange("p (j f) -> p j f", j=GT)
        nc.sync.dma_start(ch3[:, :, TD:TD + ED], e_d[grp])
        tq = gpool.tile([128, GT], F32, tag="tq")
        nc.sync.dma_start(tq[:], t_d[grp])
        dq = gpool.tile([128, GT], F32, tag="dq")
        nc.sync.dma_start(dq[:], d_d[grp])

        for wi in range(G):
            w = grp * G + wi
            hT = wpool.tile([128, 128], F32, tag="hT")
            nc.sync.dma_start(hT[:], hT_d[:, w * 128:(w + 1) * 128])
            # per-window two-zone gather with runtime descriptor counts
            idx_t = wpool.tile([128, 2 * NZS // 16], I16, tag="idx")
            nc.sync.dma_start(idx_t[:], idx_d[w])
            gat = wpool.tile([128, TW * 128], F32, tag="gat")
            # zero the slot on its first 3 uses (one per pool buf) so skipped
            # padding slots never expose non-finite stale SBUF data
            ginit = war_state.setdefault("gat_init", [0])
            if ginit[0] < 3:
                nc.vector.memset(gat[:], 0.0)
                ginit[0] += 1
            g3 = gat[:].rearrange("p (c f) -> p c f", f=128)
            if STATIC_COUNTS:
                ra, rb = NZS, NZS
            else:
                nc.gpsimd.reg_load(regA, cnt_t[0:1, 2 * w:2 * w + 1])
                nc.gpsimd.reg_load(regB, cnt_t[0:1, 2 * w + 1:2 * w + 2])
                ra, rb = regA, regB
            nc.gpsimd.dma_gather(g3[:, 0:NZT, :], src_tab[:],
                                 idx_t[:, 0:NZS // 16], num_idxs=NZS,
                                 num_idxs_reg=ra, elem_size=128,
                                 single_packet=False)
            nc.gpsimd.dma_gather(g3[:, NZT:TW, :], src_tab[HALF:, :],
                                 idx_t[:, NZS // 16:], num_idxs=NZS,
                                 num_idxs_reg=rb, elem_size=128,
                                 single_packet=False)
            pseg = ps_seg.tile([128, 384], F32)
            for j in range(TW):
                col = wi * TW + j
                # time encode: sin(freq*t + pi/2) written into chunk cols [col*128, +64)
                nc.scalar.activation(
                    chunk[:, col * 128: col * 128 + TD], freqbc_t[:],
                    mybir.ActivationFunctionType.Sin,
                    bias=halfpi_t[:], scale=tq[:, col:col + 1])
                oh = wpool.tile([128, 128], F32, tag="oh")
                nc.vector.tensor_scalar(oh[:], iota_t[:], dq[:, col:col + 1], None,
                                        op0=mybir.AluOpType.is_equal)
                # One accumulation group per PSUM bank. start/stop flags mark
                # the zero region spanned by the matmul's PARTITIONS, so the
                # group must be opened and closed by full-128-partition
                # matmuls (the M=1 count matmul goes in the middle).
                mm_src = nc.tensor.matmul(pseg[:, 0:128],
                                          gat[:, j * 128:(j + 1) * 128], oh[:],
                                          start=(j == 0), stop=False)
                if j == 0:
                    _dep_on_prev_readers(mm_src, war_state, "pseg")
                nc.tensor.matmul(pseg[0:1, 256:384], ones_col[:], oh[:],
                                 start=False, stop=False)
                mm_chk = nc.tensor.matmul(pseg[:, 128:256],
                                          chunk[:, col * 128:(col + 1) * 128], oh[:],
                                          start=False, stop=(j == TW - 1))
                if j == TW - 1:
                    seg_closer = mm_chk
            # window epilogue: inv-count + indicator, broadcast via K=1 matmul
            bc_in = wpool.tile([1, 256], F32, tag="bc_in")
            cm = wpool.tile([1, 128], F32, tag="cm")
            i_cm = nc.vector.tensor_scalar_max(cm[:], pseg[0:1, 256:384], 1.0)
            add_dep_helper(i_cm.ins, seg_closer.ins, sync=True,
                           reason="cm reads pseg bank after group close")
            nc.vector.reciprocal(bc_in[:, 0:128], cm[:])
            i_ind = nc.vector.tensor_tensor(bc_in[:, 128:256], pseg[0:1, 256:384],
                                            bc_in[:, 0:128], op=mybir.AluOpType.mult)
            bcp = ps_bc.tile([128, 256], F32)
            i_bcp = nc.tensor.matmul(bcp[:], ones_row[:], bc_in[:],
                                     start=True, stop=True)
            _dep_on_prev_readers(i_bcp, war_state, "bc")
            bcs = wpool.tile([128, 256], F32, tag="bcs")
            i_bcs = nc.scalar.copy(bcs[:], bcp[:])
            # mean chunks (transposed layout [m, n]); explicit deps on the
            # bank-group-closing matmul (readers of other regions)
            x0 = wpool.tile([128, 128], F32, tag="x0")
            i_x0 = nc.vector.tensor_tensor(x0[:], pseg[:, 0:128], bcs[:, 0:128],
                                           op=mybir.AluOpType.mult)
            add_dep_helper(i_x0.ins, seg_closer.ins, sync=True,
                           reason="x0 reads pseg bank after group close")
            x1 = wpool.tile([128, 128], F32, tag="x1")
            nc.vector.tensor_tensor(x1[:], hT[:], bcs[:, 128:256],
                                    op=mybir.AluOpType.mult)
            x2 = wpool.tile([128, 128], F32, tag="x2")
            i_x2 = nc.vector.tensor_tensor(x2[:], pseg[:, 128:256], bcs[:, 0:128],
                                           op=mybir.AluOpType.mult)
            add_dep_helper(i_x2.ins, seg_closer.ins, sync=True,
                           reason="x2 reads pseg bank after group close")
            xs = (x0, x1, x2)
            # GRU matmuls: pA regions j = gi_j (+ gh_j for j<2); pB = gh_2
            pA = ps_a.tile([128, 384], F32)
            pB = ps_b.tile([128, 128], F32)
            for jg in range(3):
                for cmi in range(3):
                    mm_a = nc.tensor.matmul(
                        pA[:, jg * 128:(jg + 1) * 128],
                        wih_t[:, (cmi * 3 + jg) * 128:(cmi * 3 + jg + 1) * 128],
                        xs[cmi][:], start=(jg == 0 and cmi == 0),
                        stop=(jg == 2 and cmi == 2))
                    if jg == 0 and cmi == 0:
                        _dep_on_prev_readers(mm_a, war_state, "pA")
                    if jg == 2 and cmi == 2:
                        a_closer = mm_a
                if jg < 2:
                    nc.tensor.matmul(
                        pA[:, jg * 128:(jg + 1) * 128],
                        whh_t[:, jg * 128:(jg + 1) * 128], hT[:],
                        start=False, stop=False)
            i_pb = nc.tensor.matmul(pB[:], whh_t[:, 256:384], hT[:],
                                    start=True, stop=True)
            _dep_on_prev_readers(i_pb, war_state, "pB")
            # gates
            r = wpool.tile([128, 128], F32, tag="r")
            i_r = nc.scalar.activation(r[:], pA[:, 0:128],
                                       mybir.ActivationFunctionType.Sigmoid,
                                       bias=bias4[:, 0:1])
            add_dep_helper(i_r.ins, a_closer.ins, sync=True,
                           reason="r reads pA bank after group close")
            z = wpool.tile([128, 128], F32, tag="z")
            i_z = nc.scalar.activation(z[:], pA[:, 128:256],
                                       mybir.ActivationFunctionType.Sigmoid,
                                       bias=bias4[:, 1:2])
            add_dep_helper(i_z.ins, a_closer.ins, sync=True,
                           reason="z reads pA bank after group close")
            v1 = wpool.tile([128, 128], F32, tag="v1")
            i_v1 = nc.vector.tensor_scalar_add(v1[:], pB[:], bias4[:, 3:4])
            v2 = wpool.tile([128, 128], F32, tag="v2")
            nc.vector.tensor_tensor(v2[:], v1[:], r[:], op=mybir.AluOpType.mult)
            v3 = wpool.tile([128, 128], F32, tag="v3")
            i_v3 = nc.vector.tensor_tensor(v3[:], v2[:], pA[:, 256:384],
                                           op=mybir.AluOpType.add)
            ngate = wpool.tile([128, 128], F32, tag="n")
            nc.scalar.activation(ngate[:], v3[:],
                                 mybir.ActivationFunctionType.Tanh,
                                 bias=bias4[:, 2:3])
            d1 = wpool.tile([128, 128], F32, tag="d1")
            nc.vector.tensor_tensor(d1[:], hT[:], ngate[:],
                                    op=mybir.AluOpType.subtract)
            zd = wpool.tile([128, 128], F32, tag="zd")
            nc.vector.tensor_tensor(zd[:], z[:], d1[:], op=mybir.AluOpType.mult)
            o = wpool.tile([128, 128], F32, tag="o")
            nc.vector.tensor_tensor(o[:], ngate[:], zd[:], op=mybir.AluOpType.add)
            nc.sync.dma_start(out_d[:, w * 128:(w + 1) * 128], o[:])
            # record psum readers of this window for WAR enforcement
            war_state.setdefault("pseg", []).append([i_x0, i_x2, i_cm, i_ind])
            war_state.setdefault("bc", []).append([i_bcs])
            war_state.setdefault("pA", []).append([i_r, i_z, i_v3])
            war_state.setdefault("pB", []).append([i_v1])


def _build():
    nc = bacc.Bacc("TRN2", target_bir_lowering=False, debug=False)

    si_full = nc.dram_tensor("si_full", [NPAD, S], F32, kind="ExternalInput")
    sir_full = nc.dram_tensor("sir_full", [NPAD, S], F32, kind="ExternalInput")
    hTg = nc.dram_tensor("hTg", [S, NPC], F32, kind="ExternalInput")
    hTr = nc.dram_tensor("hTr", [S, NPC], F32, kind="ExternalInput")
    io = {}
    for nm in ("g", "r"):
        io["idx_" + nm] = nc.dram_tensor("idx_" + nm, [NW, 128, 2 * NZS // 16],
                                         I16, kind="ExternalInput")
        io["cnt_" + nm] = nc.dram_tensor("cnt_" + nm, [1, 2 * NW], I32,
                                         kind="ExternalInput")
        io["e_" + nm] = nc.dram_tensor("e_" + nm, [NG, 128, GT, ED], F32,
                                       kind="ExternalInput")
        io["t_" + nm] = nc.dram_tensor("t_" + nm, [NG, 128, GT], F32,
                                       kind="ExternalInput")
        io["d_" + nm] = nc.dram_tensor("d_" + nm, [NG, 128, GT], F32,
                                       kind="ExternalInput")
        io["outT_" + nm] = nc.dram_tensor("outT_" + nm, [S, NPC], F32,
                                          kind="ExternalOutput")
    wih = nc.dram_tensor("wih_t", [128, 9 * 128], F32, kind="ExternalInput")
    whh = nc.dram_tensor("whh_t", [S, 384], F32, kind="ExternalInput")
    bias4 = nc.dram_tensor("bias4", [128, 4], F32, kind="ExternalInput")
    iota_d = nc.dram_tensor("iota", [128, 128], F32, kind="ExternalInput")
    freqbc_d = nc.dram_tensor("freqbc", [128, TD], F32, kind="ExternalInput")

    with tile.TileContext(nc) as tc:
        with (
            tc.tile_pool(name="cst", bufs=1) as cpool,
            tc.tile_pool(name="grp", bufs=2) as gpool,
            tc.tile_pool(name="win", bufs=3) as wpool,
            tc.tile_pool(name="ps_seg", bufs=2, space="PSUM") as ps_seg,
            tc.tile_pool(name="ps_bc", bufs=2, space="PSUM") as ps_bc,
            tc.tile_pool(name="ps_a", bufs=2, space="PSUM") as ps_a,
            tc.tile_pool(name="ps_b", bufs=2, space="PSUM") as ps_b,
        ):
            wih_t = cpool.tile([128, 9 * 128], F32)
            nc.sync.dma_start(wih_t[:], wih[:])
            whh_t = cpool.tile([S, 384], F32)
            nc.sync.dma_start(whh_t[:], whh[:])
            bias_t = cpool.tile([128, 4], F32)
            nc.sync.dma_start(bias_t[:], bias4[:])
            iota_t = cpool.tile([128, 128], F32)
            nc.sync.dma_start(iota_t[:], iota_d[:])
            freqbc_t = cpool.tile([128, TD], F32)
            nc.sync.dma_start(freqbc_t[:], freqbc_d[:])
            halfpi_t = cpool.tile([128, 1], F32)
            nc.vector.memset(halfpi_t[:], float(np.pi / 2))
            ones_col = cpool.tile([128, 1], F32)
            nc.vector.memset(ones_col[:], 1.0)
            ones_row = cpool.tile([1, 128], F32)
            nc.vector.memset(ones_row[:], 1.0)
            regA = nc.gpsimd.alloc_register("gath_cnt_a")
            regB = nc.gpsimd.alloc_register("gath_cnt_b")
            pools = (gpool, wpool, ps_seg, ps_bc, ps_a, ps_b)
            cst = (wih_t, whh_t, bias_t, iota_t, freqbc_t, halfpi_t,
                   ones_col, ones_row)
            war_state = {}
            _emit_graph(nc, tc, pools, cst,
                        (si_full, hTg, io["idx_g"], io["cnt_g"], io["e_g"],
                         io["t_g"], io["d_g"], io["outT_g"]),
                        war_state, (regA, regB))
            _emit_graph(nc, tc, pools, cst,
                        (sir_full, hTr, io["idx_r"], io["cnt_r"], io["e_r"],
                         io["t_r"], io["d_r"], io["outT_r"]),
                        war_state, (regA, regB))
    nc.compile()
    return nc


def _wrap_idx(vals):
    """int16 index list -> dma_gather SBUF layout [128, NZS//16]:
    idx i at [i%16 + 16*replica, i//16], replicated for the 8 Q7 cores."""
    full = np.full(NZS, 0 if STATIC_COUNTS else -1, np.int16)
    full[:len(vals)] = vals.astype(np.int16)
    blk = full.reshape(NZS // 16, 16).T  # [16, 48]
    return np.tile(blk, (8, 1))


def _prep_graph(src, dst, t, e):
    """Sort by dst, shard by dst range across cores, split each window's
    edges into two src zones (int16 gather limit), pack into tile slots."""
    order = np.argsort(dst, kind='stable')
    ds = dst[order].astype(np.int64)
    ss = src[order].astype(np.int64)
    ts = t[order].astype(np.float32)
    es = e[order].astype(np.float32)
    wb = np.searchsorted(ds, np.arange(0, NPAD + 1, 128))
    idx_a = np.zeros((NCORES, NW, 128, 2 * NZS // 16), np.int16)
    cnt_a = np.ones((NCORES, 1, 2 * NW), np.int32)
    slot_a = np.zeros((NCORES, NG, 128, GT), np.int64)  # for emulation
    d_a = np.full((NCORES, NG, 128, GT), 200.0, np.float32)
    t_a = np.zeros((NCORES, NG, 128, GT), np.float32)
    e_a = np.zeros((NCORES, NG, 128, GT, ED), np.float32)
    for c in range(NCORES):
        for w in range(NW):
            gw = c * NW + w
            lo, hi = int(wb[gw]), int(wb[gw + 1])
            grp, wi = w // G, w % G
            sseg = ss[lo:hi]
            za = np.nonzero(sseg < HALF)[0]
            zb = np.nonzero(sseg >= HALF)[0]
            assert len(za) <= NZS and len(zb) <= NZS, \
                f"zone overflow: {len(za)}/{len(zb)}"
            idx_a[c, w, :, :NZS // 16] = _wrap_idx(sseg[za])
            idx_a[c, w, :, NZS // 16:] = _wrap_idx(sseg[zb] - HALF)
            cnt_a[c, 0, 2 * w] = max(len(za), 1)
            cnt_a[c, 0, 2 * w + 1] = max(len(zb), 1)
            for zi, zz in ((0, za), (1, zb)):
                n = len(zz)
                if n == 0:
                    continue
                k = np.arange(n)
                p = k % 128
                cols = wi * TW + zi * NZT + k // 128
                sel = lo + zz
                slot_a[c, grp, p, cols] = ss[sel]
                d_a[c, grp, p, cols] = (ds[sel] - gw * 128).astype(np.float32)
                t_a[c, grp, p, cols] = ts[sel]
                e_a[c, grp, p, cols, :] = es[sel]
    return idx_a, cnt_a, slot_a, d_a, t_a, e_a


last_results = None


def kernel(si, sj, si_r, sj_r, t, t_r, e, e_r,
           w_ih, w_hh, b_ih, b_hh,
           src_g, dst_g, src_gr, dst_gr):
    global last_results
    si = np.asarray(si, np.float32)
    sj = np.asarray(sj, np.float32)
    si_r = np.asarray(si_r, np.float32)
    sj_r = np.asarray(sj_r, np.float32)

    if "nc" not in _CACHE:
        _CACHE["nc"] = _build()
    nc = _CACHE["nc"]

    def padrows(a):
        out = np.zeros((NPAD, S), np.float32)
        out[:a.shape[0]] = a
        return out

    si_p = padrows(si)
    sir_p = padrows(si_r)
    sj_p = padrows(sj)
    sjr_p = padrows(sj_r)

    ig, cg, _sg, dg, tg, eg = _prep_graph(np.asarray(src_g), np.asarray(dst_g),
                                          np.asarray(t), np.asarray(e))
    ir_, cr, _sr, dr, tr, er = _prep_graph(np.asarray(src_gr), np.asarray(dst_gr),
                                           np.asarray(t_r), np.asarray(e_r))

    wih_t = np.ascontiguousarray(np.asarray(w_ih, np.float32).T)  # [384 m, 384 g]
    wih_host = np.zeros((128, 9 * 128), np.float32)
    for cm in range(3):
        for jg in range(3):
            wih_host[:, (cm * 3 + jg) * 128:(cm * 3 + jg + 1) * 128] = \
                wih_t[cm * 128:(cm + 1) * 128, jg * 128:(jg + 1) * 128]
    whh_host = np.ascontiguousarray(np.asarray(w_hh, np.float32).T)  # [128, 384]
    b_ih = np.asarray(b_ih, np.float32)
    b_hh = np.asarray(b_hh, np.float32)
    bias_host = np.stack([
        (b_ih + b_hh)[0:128], (b_ih + b_hh)[128:256],
        b_ih[256:384], b_hh[256:384]], axis=1).astype(np.float32)
    iota_host = np.tile(np.arange(128, dtype=np.float32), (128, 1))
    freq = (1.0 / 10 ** np.linspace(0, 9, TD)).astype(np.float32)
    freqbc_host = np.tile(freq, (128, 1))

    in_maps = []
    for c in range(NCORES):
        sl = slice(c * NPC, (c + 1) * NPC)
        in_maps.append({
            "si_full": si_p, "sir_full": sir_p,
            "hTg": np.ascontiguousarray(sj_p[sl].T),
            "hTr": np.ascontiguousarray(sjr_p[sl].T),
            "idx_g": ig[c], "cnt_g": cg[c], "e_g": eg[c], "t_g": tg[c],
            "d_g": dg[c],
            "idx_r": ir_[c], "cnt_r": cr[c], "e_r": er[c], "t_r": tr[c],
            "d_r": dr[c],
            "wih_t": wih_host, "whh_t": whh_host, "bias4": bias_host,
            "iota": iota_host, "freqbc": freqbc_host,
        })

    res = bass_utils.run_bass_kernel_spmd(nc, in_maps, core_ids=list(range(NCORES)))
    last_results = res

    sj_out = np.concatenate([res.results[c]["outT_g"].T for c in range(NCORES)],
                            axis=0)[:NN]
    si_out = np.concatenate([res.results[c]["outT_r"].T for c in range(NCORES)],
                            axis=0)[:NN]
    return (si_out, sj_out)


# revision 36
# speedup vs baseline: 1.1592x; 1.1592x over previous
"""TGN message-passing kernel for 8 Trainium2 NeuronCores.

Strategy:
  - Sort edges by destination; shard edges across cores by destination node
    range (6272 nodes/core). No collectives needed.
  - Per 128-node window: segment-sum via one-hot matmuls accumulated in PSUM
    (src-memory chunk + [timeenc|edgefeat] chunk + count row), then the
    dst-memory chunk analytically = count[n] * dst_mem[n] (no gather).
  - GRU fused per window: gi/gh matmuls accumulate in PSUM, gates on ACT/DVE,
    output written transposed; host re-transposes and trims.
"""
import os
import sys
sys.path.insert(0, '/opt/trn_rl_repo')
import numpy as np
from concourse import bass, bacc, tile, mybir
from concourse import bass_utils
from concourse.tile import add_dep_helper

F32 = mybir.dt.float32
I32 = mybir.dt.int32
I16 = mybir.dt.int16

NN = 50000      # nodes per side (users == items == 50000)
EDG = 400000
S = 128         # memory dim
TD = 64         # time-encode dim
ED = 64         # edge-feat dim
NCORES = 8
NW = 49         # windows per core per graph
NPC = NW * 128  # 6272 nodes per core
NPAD = NPC * NCORES  # 50176
NZT = 6         # tiles per src zone (zone A: src < HALF, zone B: src >= HALF)
TW = 2 * NZT    # 12 edge tiles (of 128) per window
NZS = NZT * 128  # 768 slots per zone
HALF = 25088    # src-table zone split (int16 gather index limit)
G = 7           # windows per e/t/d batch group
NG = NW // G    # 7 groups
GT = G * TW     # 84 tiles per group

_CACHE = {}


def _dep_on_prev_readers(inst, state, tag, bufs=2):
    """Sync the first writer of a reused PSUM slot on the readers of the
    tile that previously occupied it (Tile's WAR enforcement misses this)."""
    hist = state.setdefault(tag, [])
    if len(hist) >= bufs:
        for rd in hist[-bufs]:
            add_dep_helper(inst.ins, rd.ins, sync=True,
                           reason=f"WAR: {tag} slot reuse")


def _emit_graph(nc, tc, pools, cst, tensors, war_state, cq):
    """Emit one graph's 49 windows. cq[w] = (countA, countB) static
    descriptor counts (max over cores, quantized)."""
    (gpool, wpool, ps_seg, ps_bc, ps_a, ps_b) = pools
    (wih_t, whh_t, bias4, iota_t, freqbc_t, halfpi_t, ones_col, ones_row) = cst
    (src_tab, hT_d, idx_d, e_d, t_d, d_d, out_d) = tensors

    for grp in range(NG):
        chunk = gpool.tile([128, GT * 128], F32, tag="chunk")
        ch3 = chunk[:].rearrange("p (j f) -> p j f", j=GT)
        nc.sync.dma_start(ch3[:, :, TD:TD + ED], e_d[grp])
        tq = gpool.tile([128, GT], F32, tag="tq")
        nc.sync.dma_start(tq[:], t_d[grp])
        dq = gpool.tile([128, GT], F32, tag="dq")
        nc.sync.dma_start(dq[:], d_d[grp])

        for wi in range(G):
            w = grp * G + wi
            hT = wpool.tile([128, 128], F32, tag="hT")
            nc.sync.dma_start(hT[:], hT_d[:, w * 128:(w + 1) * 128])
            # per-window two-zone gather with runtime descriptor counts
            idx_t = wpool.tile([128, 2 * NZS // 16], I16, tag="idx")
            nc.sync.dma_start(idx_t[:], idx_d[w])
            gat = wpool.tile([128, TW * 128], F32, tag="gat")
            # zero the slot on its first 3 uses (one per pool buf) so skipped
            # padding slots never expose non-finite stale SBUF data
            ginit = war_state.setdefault("gat_init", [0])
            if ginit[0] < 3:
                nc.vector.memset(gat[:], 0.0)
                ginit[0] += 1
            g3 = gat[:].rearrange("p (c f) -> p c f", f=128)
            ra, rb = int(cq[w][0]), int(cq[w][1])
            nc.gpsimd.dma_gather(g3[:, 0:NZT, :], src_tab[:],
                                 idx_t[:, 0:NZS // 16], num_idxs=NZS,
                                 num_idxs_reg=ra, elem_size=128,
                                 single_packet=False)
            nc.gpsimd.dma_gather(g3[:, NZT:TW, :], src_tab[HALF:, :],
                                 idx_t[:, NZS // 16:], num_idxs=NZS,
                                 num_idxs_reg=rb, elem_size=128,
                                 single_packet=False)
            pseg = ps_seg.tile([128, 384], F32)
            for j in range(TW):
                col = wi * TW + j
                # time encode: sin(freq*t + pi/2) written into chunk cols [col*128, +64)
                nc.scalar.activation(
                    chunk[:, col * 128: col * 128 + TD], freqbc_t[:],
                    mybir.ActivationFunctionType.Sin,
                    bias=halfpi_t[:], scale=tq[:, col:col + 1])
                oh = wpool.tile([128, 128], F32, tag="oh")
                nc.vector.tensor_scalar(oh[:], iota_t[:], dq[:, col:col + 1], None,
                                        op0=mybir.AluOpType.is_equal)
                # One accumulation group per PSUM bank. start/stop flags mark
                # the zero region spanned by the matmul's PARTITIONS, so the
                # group must be opened and closed by full-128-partition
                # matmuls (the M=1 count matmul goes in the middle).
                mm_src = nc.tensor.matmul(pseg[:, 0:128],
                                          gat[:, j * 128:(j + 1) * 128], oh[:],
                                          start=(j == 0), stop=False)
                if j == 0:
                    _dep_on_prev_readers(mm_src, war_state, "pseg")
                nc.tensor.matmul(pseg[0:1, 256:384], ones_col[:], oh[:],
                                 start=False, stop=False)
                mm_chk = nc.tensor.matmul(pseg[:, 128:256],
                                          chunk[:, col * 128:(col + 1) * 128], oh[:],
                                          start=False, stop=(j == TW - 1))
                if j == TW - 1:
                    seg_closer = mm_chk
            # window epilogue: inv-count + indicator, broadcast via K=1 matmul
            bc_in = wpool.tile([1, 256], F32, tag="bc_in")
            cm = wpool.tile([1, 128], F32, tag="cm")
            i_cm = nc.vector.tensor_scalar_max(cm[:], pseg[0:1, 256:384], 1.0)
            add_dep_helper(i_cm.ins, seg_closer.ins, sync=True,
                           reason="cm reads pseg bank after group close")
            nc.vector.reciprocal(bc_in[:, 0:128], cm[:])
            i_ind = nc.vector.tensor_tensor(bc_in[:, 128:256], pseg[0:1, 256:384],
                                            bc_in[:, 0:128], op=mybir.AluOpType.mult)
            bcp = ps_bc.tile([128, 256], F32)
            i_bcp = nc.tensor.matmul(bcp[:], ones_row[:], bc_in[:],
                                     start=True, stop=True)
            _dep_on_prev_readers(i_bcp, war_state, "bc")
            bcs = wpool.tile([128, 256], F32, tag="bcs")
            i_bcs = nc.scalar.copy(bcs[:], bcp[:])
            # mean chunks (transposed layout [m, n]); explicit deps on the
            # bank-group-closing matmul (readers of other regions)
            x0 = wpool.tile([128, 128], F32, tag="x0")
            i_x0 = nc.vector.tensor_tensor(x0[:], pseg[:, 0:128], bcs[:, 0:128],
                                           op=mybir.AluOpType.mult)
            add_dep_helper(i_x0.ins, seg_closer.ins, sync=True,
                           reason="x0 reads pseg bank after group close")
            x1 = wpool.tile([128, 128], F32, tag="x1")
            nc.vector.tensor_tensor(x1[:], hT[:], bcs[:, 128:256],
                                    op=mybir.AluOpType.mult)
            x2 = wpool.tile([128, 128], F32, tag="x2")
            i_x2 = nc.vector.tensor_tensor(x2[:], pseg[:, 128:256], bcs[:, 0:128],
                                           op=mybir.AluOpType.mult)
            add_dep_helper(i_x2.ins, seg_closer.ins, sync=True,
                           reason="x2 reads pseg bank after group close")
            xs = (x0, x1, x2)
            # GRU matmuls: pA regions j = gi_j (+ gh_j for j<2); pB = gh_2
            pA = ps_a.tile([128, 384], F32)
            pB = ps_b.tile([128, 128], F32)
            for jg in range(3):
                for cmi in range(3):
                    mm_a = nc.tensor.matmul(
                        pA[:, jg * 128:(jg + 1) * 128],
                        wih_t[:, (cmi * 3 + jg) * 128:(cmi * 3 + jg + 1) * 128],
                        xs[cmi][:], start=(jg == 0 and cmi == 0),
                        stop=(jg == 2 and cmi == 2))
                    if jg == 0 and cmi == 0:
                        _dep_on_prev_readers(mm_a, war_state, "pA")
                    if jg == 2 and cmi == 2:
                        a_closer = mm_a
                if jg < 2:
                    nc.tensor.matmul(
                        pA[:, jg * 128:(jg + 1) * 128],
                        whh_t[:, jg * 128:(jg + 1) * 128], hT[:],
                        start=False, stop=False)
            i_pb = nc.tensor.matmul(pB[:], whh_t[:, 256:384], hT[:],
                                    start=True, stop=True)
            _dep_on_prev_readers(i_pb, war_state, "pB")
            # gates
            r = wpool.tile([128, 128], F32, tag="r")
            i_r = nc.scalar.activation(r[:], pA[:, 0:128],
                                       mybir.ActivationFunctionType.Sigmoid,
                                       bias=bias4[:, 0:1])
            add_dep_helper(i_r.ins, a_closer.ins, sync=True,
                           reason="r reads pA bank after group close")
            z = wpool.tile([128, 128], F32, tag="z")
            i_z = nc.scalar.activation(z[:], pA[:, 128:256],
                                       mybir.ActivationFunctionType.Sigmoid,
                                       bias=bias4[:, 1:2])
            add_dep_helper(i_z.ins, a_closer.ins, sync=True,
                           reason="z reads pA bank after group close")
            v1 = wpool.tile([128, 128], F32, tag="v1")
            i_v1 = nc.vector.tensor_scalar_add(v1[:], pB[:], bias4[:, 3:4])
            v2 = wpool.tile([128, 128], F32, tag="v2")
            nc.vector.tensor_tensor(v2[:], v1[:], r[:], op=mybir.AluOpType.mult)
            v3 = wpool.tile([128, 128], F32, tag="v3")
            i_v3 = nc.vector.tensor_tensor(v3[:], v2[:], pA[:, 256:384],
                                           op=mybir.AluOpType.add)
            ngate = wpool.tile([128, 128], F32, tag="n")
            nc.scalar.activation(ngate[:], v3[:],
                                 mybir.ActivationFunctionType.Tanh,
                                 bias=bias4[:, 2:3])
            d1 = wpool.tile([128, 128], F32, tag="d1")
            nc.vector.tensor_tensor(d1[:], hT[:], ngate[:],
                                    op=mybir.AluOpType.subtract)
            zd = wpool.tile([128, 128], F32, tag="zd")
            nc.vector.tensor_tensor(zd[:], z[:], d1[:], op=mybir.AluOpType.mult)
            o = wpool.tile([128, 128], F32, tag="o")
            nc.vector.tensor_tensor(o[:], ngate[:], zd[:], op=mybir.AluOpType.add)
            nc.sync.dma_start(out_d[:, w * 128:(w + 1) * 128], o[:])
            # record psum readers of this window for WAR enforcement
            war_state.setdefault("pseg", []).append([i_x0, i_x2, i_cm, i_ind])
            war_state.setdefault("bc", []).append([i_bcs])
            war_state.setdefault("pA", []).append([i_r, i_z, i_v3])
            war_state.setdefault("pB", []).append([i_v1])


def _build(cq_g, cq_r):
    nc = bacc.Bacc("TRN2", target_bir_lowering=False, debug=False)

    si_full = nc.dram_tensor("si_full", [NPAD, S], F32, kind="ExternalInput")
    sir_full = nc.dram_tensor("sir_full", [NPAD, S], F32, kind="ExternalInput")
    hTg = nc.dram_tensor("hTg", [S, NPC], F32, kind="ExternalInput")
    hTr = nc.dram_tensor("hTr", [S, NPC], F32, kind="ExternalInput")
    io = {}
    for nm in ("g", "r"):
        io["idx_" + nm] = nc.dram_tensor("idx_" + nm, [NW, 128, 2 * NZS // 16],
                                         I16, kind="ExternalInput")
        io["e_" + nm] = nc.dram_tensor("e_" + nm, [NG, 128, GT, ED], F32,
                                       kind="ExternalInput")
        io["t_" + nm] = nc.dram_tensor("t_" + nm, [NG, 128, GT], F32,
                                       kind="ExternalInput")
        io["d_" + nm] = nc.dram_tensor("d_" + nm, [NG, 128, GT], F32,
                                       kind="ExternalInput")
        io["outT_" + nm] = nc.dram_tensor("outT_" + nm, [S, NPC], F32,
                                          kind="ExternalOutput")
    wih = nc.dram_tensor("wih_t", [128, 9 * 128], F32, kind="ExternalInput")
    whh = nc.dram_tensor("whh_t", [S, 384], F32, kind="ExternalInput")
    bias4 = nc.dram_tensor("bias4", [128, 4], F32, kind="ExternalInput")
    iota_d = nc.dram_tensor("iota", [128, 128], F32, kind="ExternalInput")
    freqbc_d = nc.dram_tensor("freqbc", [128, TD], F32, kind="ExternalInput")

    with tile.TileContext(nc) as tc:
        with (
            tc.tile_pool(name="cst", bufs=1) as cpool,
            tc.tile_pool(name="grp", bufs=2) as gpool,
            tc.tile_pool(name="win", bufs=3) as wpool,
            tc.tile_pool(name="ps_seg", bufs=2, space="PSUM") as ps_seg,
            tc.tile_pool(name="ps_bc", bufs=2, space="PSUM") as ps_bc,
            tc.tile_pool(name="ps_a", bufs=2, space="PSUM") as ps_a,
            tc.tile_pool(name="ps_b", bufs=2, space="PSUM") as ps_b,
        ):
            wih_t = cpool.tile([128, 9 * 128], F32)
            nc.sync.dma_start(wih_t[:], wih[:])
            whh_t = cpool.tile([S, 384], F32)
            nc.sync.dma_start(whh_t[:], whh[:])
            bias_t = cpool.tile([128, 4], F32)
            nc.sync.dma_start(bias_t[:], bias4[:])
            iota_t = cpool.tile([128, 128], F32)
            nc.sync.dma_start(iota_t[:], iota_d[:])
            freqbc_t = cpool.tile([128, TD], F32)
            nc.sync.dma_start(freqbc_t[:], freqbc_d[:])
            halfpi_t = cpool.tile([128, 1], F32)
            nc.vector.memset(halfpi_t[:], float(np.pi / 2))
            ones_col = cpool.tile([128, 1], F32)
            nc.vector.memset(ones_col[:], 1.0)
            ones_row = cpool.tile([1, 128], F32)
            nc.vector.memset(ones_row[:], 1.0)
            pools = (gpool, wpool, ps_seg, ps_bc, ps_a, ps_b)
            cst = (wih_t, whh_t, bias_t, iota_t, freqbc_t, halfpi_t,
                   ones_col, ones_row)
            war_state = {}
            _emit_graph(nc, tc, pools, cst,
                        (si_full, hTg, io["idx_g"], io["e_g"],
                         io["t_g"], io["d_g"], io["outT_g"]),
                        war_state, cq_g)
            _emit_graph(nc, tc, pools, cst,
                        (sir_full, hTr, io["idx_r"], io["e_r"],
                         io["t_r"], io["d_r"], io["outT_r"]),
                        war_state, cq_r)
    nc.compile()
    return nc


def _wrap_idx(vals, nvalid):
    """int16 index list (0-padded to nvalid, -1 beyond) -> dma_gather SBUF
    layout [128, NZS//16]: idx i at [i%16 + 16*replica, i//16], replicated
    for the 8 Q7 cores."""
    full = np.full(NZS, -1, np.int16)
    full[:nvalid] = 0
    full[:len(vals)] = vals.astype(np.int16)
    blk = full.reshape(NZS // 16, 16).T  # [16, 48]
    return np.tile(blk, (8, 1))


QUANT = 64  # descriptor-count quantum (static counts shared across cores)


def _prep_graph(src, dst, t, e):
    """Sort by dst, shard by dst range across cores, split each window's
    edges into two src zones (int16 gather limit), pack into tile slots.
    Returns per-core arrays plus static quantized per-window counts."""
    order = np.argsort(dst, kind='stable')
    ds = dst[order].astype(np.int64)
    ss = src[order].astype(np.int64)
    ts = t[order].astype(np.float32)
    es = e[order].astype(np.float32)
    wb = np.searchsorted(ds, np.arange(0, NPAD + 1, 128))
    zidx = {}   # (c, w, zone) -> index values
    slot_a = np.zeros((NCORES, NG, 128, GT), np.int64)  # for emulation
    d_a = np.full((NCORES, NG, 128, GT), 200.0, np.float32)
    t_a = np.zeros((NCORES, NG, 128, GT), np.float32)
    e_a = np.zeros((NCORES, NG, 128, GT, ED), np.float32)
    for c in range(NCORES):
        for w in range(NW):
            gw = c * NW + w
            lo, hi = int(wb[gw]), int(wb[gw + 1])
            grp, wi = w // G, w % G
            sseg = ss[lo:hi]
            za = np.nonzero(sseg < HALF)[0]
            zb = np.nonzero(sseg >= HALF)[0]
            assert len(za) <= NZS and len(zb) <= NZS, \
                f"zone overflow: {len(za)}/{len(zb)}"
            zidx[(c, w, 0)] = sseg[za]
            zidx[(c, w, 1)] = sseg[zb] - HALF
            for zi, zz in ((0, za), (1, zb)):
                n = len(zz)
                if n == 0:
                    continue
                k = np.arange(n)
                p = k % 128
                cols = wi * TW + zi * NZT + k // 128
                sel = lo + zz
                slot_a[c, grp, p, cols] = ss[sel]
                d_a[c, grp, p, cols] = (ds[sel] - gw * 128).astype(np.float32)
                t_a[c, grp, p, cols] = ts[sel]
                e_a[c, grp, p, cols, :] = es[sel]
    # static counts: max over cores, quantized up
    cq = []
    for w in range(NW):
        row = []
        for zi in range(2):
            m = max(len(zidx[(c, w, zi)]) for c in range(NCORES))
            row.append(min(NZS, max(QUANT, -(-m // QUANT) * QUANT)))
        cq.append(tuple(row))
    idx_a = np.zeros((NCORES, NW, 128, 2 * NZS // 16), np.int16)
    for c in range(NCORES):
        for w in range(NW):
            idx_a[c, w, :, :NZS // 16] = _wrap_idx(zidx[(c, w, 0)], cq[w][0])
            idx_a[c, w, :, NZS // 16:] = _wrap_idx(zidx[(c, w, 1)], cq[w][1])
    return idx_a, cq, slot_a, d_a, t_a, e_a


last_results = None


def kernel(si, sj, si_r, sj_r, t, t_r, e, e_r,
           w_ih, w_hh, b_ih, b_hh,
           src_g, dst_g, src_gr, dst_gr):
    global last_results
    si = np.asarray(si, np.float32)
    sj = np.asarray(sj, np.float32)
    si_r = np.asarray(si_r, np.float32)
    sj_r = np.asarray(sj_r, np.float32)

    def padrows(a):
        out = np.zeros((NPAD, S), np.float32)
        out[:a.shape[0]] = a
        return out

    si_p = padrows(si)
    sir_p = padrows(si_r)
    sj_p = padrows(sj)
    sjr_p = padrows(sj_r)

    ig, cg, _sg, dg, tg, eg = _prep_graph(np.asarray(src_g), np.asarray(dst_g),
                                          np.asarray(t), np.asarray(e))
    ir_, cr, _sr, dr, tr, er = _prep_graph(np.asarray(src_gr), np.asarray(dst_gr),
                                           np.asarray(t_r), np.asarray(e_r))

    key = (tuple(cg), tuple(cr))
    if _CACHE.get("key") != key:
        _CACHE["nc"] = _build(cg, cr)
        _CACHE["key"] = key
    nc = _CACHE["nc"]

    wih_t = np.ascontiguousarray(np.asarray(w_ih, np.float32).T)  # [384 m, 384 g]
    wih_host = np.zeros((128, 9 * 128), np.float32)
    for cm in range(3):
        for jg in range(3):
            wih_host[:, (cm * 3 + jg) * 128:(cm * 3 + jg + 1) * 128] = \
                wih_t[cm * 128:(cm + 1) * 128, jg * 128:(jg + 1) * 128]
    whh_host = np.ascontiguousarray(np.asarray(w_hh, np.float32).T)  # [128, 384]
    b_ih = np.asarray(b_ih, np.float32)
    b_hh = np.asarray(b_hh, np.float32)
    bias_host = np.stack([
        (b_ih + b_hh)[0:128], (b_ih + b_hh)[128:256],
        b_ih[256:384], b_hh[256:384]], axis=1).astype(np.float32)
    iota_host = np.tile(np.arange(128, dtype=np.float32), (128, 1))
    freq = (1.0 / 10 ** np.linspace(0, 9, TD)).astype(np.float32)
    freqbc_host = np.tile(freq, (128, 1))

    in_maps = []
    for c in range(NCORES):
        sl = slice(c * NPC, (c + 1) * NPC)
        in_maps.append({
            "si_full": si_p, "sir_full": sir_p,
            "hTg": np.ascontiguousarray(sj_p[sl].T),
            "hTr": np.ascontiguousarray(sjr_p[sl].T),
            "idx_g": ig[c], "e_g": eg[c], "t_g": tg[c], "d_g": dg[c],
            "idx_r": ir_[c], "e_r": er[c], "t_r": tr[c], "d_r": dr[c],
            "wih_t": wih_host, "whh_t": whh_host, "bias4": bias_host,
            "iota": iota_host, "freqbc": freqbc_host,
        })

    res = bass_utils.run_bass_kernel_spmd(nc, in_maps, core_ids=list(range(NCORES)))
    last_results = res

    sj_out = np.concatenate([res.results[c]["outT_g"].T for c in range(NCORES)],
                            axis=0)[:NN]
    si_out = np.concatenate([res.results[c]["outT_r"].T for c in range(NCORES)],
                            axis=0)[:NN]
    return (si_out, sj_out)


# revision 43
# speedup vs baseline: 1.2245x; 1.0564x over previous
"""TGN message-passing kernel for 8 Trainium2 NeuronCores.

Strategy:
  - Sort edges by destination; shard edges across cores by destination node
    range (6272 nodes/core). No collectives needed.
  - Per 128-node window: segment-sum via one-hot matmuls accumulated in PSUM
    (src-memory chunk + [timeenc|edgefeat] chunk + count row), then the
    dst-memory chunk analytically = count[n] * dst_mem[n] (no gather).
  - GRU fused per window: gi/gh matmuls accumulate in PSUM, gates on ACT/DVE,
    output written transposed; host re-transposes and trims.
"""
import os
import sys
sys.path.insert(0, '/opt/trn_rl_repo')
import ml_dtypes
import numpy as np
from concourse import bass, bacc, tile, mybir
from concourse import bass_utils
from concourse.tile import add_dep_helper

F32 = mybir.dt.float32
BF = mybir.dt.bfloat16
I32 = mybir.dt.int32
I16 = mybir.dt.int16

NN = 50000      # nodes per side (users == items == 50000)
EDG = 400000
S = 128         # memory dim
TD = 64         # time-encode dim
ED = 64         # edge-feat dim
NCORES = 8
NW = 49         # windows per core per graph
NPC = NW * 128  # 6272 nodes per core
NPAD = NPC * NCORES  # 50176
NZT = 6         # tiles per src zone (zone A: src < HALF, zone B: src >= HALF)
TW = 2 * NZT    # 12 edge tiles (of 128) per window
NZS = NZT * 128  # 768 slots per zone
HALF = 25088    # src-table zone split (int16 gather index limit)
G = 7           # windows per e/t/d batch group
NG = NW // G    # 7 groups
GT = G * TW     # 84 tiles per group

_CACHE = {}


def _dep_on_prev_readers(inst, state, tag, bufs=2):
    """Sync the first writer of a reused PSUM slot on the readers of the
    tile that previously occupied it (Tile's WAR enforcement misses this)."""
    hist = state.setdefault(tag, [])
    if len(hist) >= bufs:
        for rd in hist[-bufs]:
            add_dep_helper(inst.ins, rd.ins, sync=True,
                           reason=f"WAR: {tag} slot reuse")


def _emit_graph(nc, tc, pools, cst, tensors, war_state, cq):
    """Emit one graph's 49 windows. cq[w] = (countA, countB) static
    descriptor counts (max over cores, quantized)."""
    (gpool, wpool, ps_seg, ps_bc, ps_a, ps_b) = pools
    (wih_t, whh_t, bias4, iota_t, freqbc_t, halfpi_t, ones_col, ones_row) = cst
    (src_tab, hT_d, idx_d, e_d, t_d, d_d, out_d) = tensors

    for grp in range(NG):
        chunk = gpool.tile([128, GT * 128], BF, tag="chunk")
        ch3 = chunk[:].rearrange("p (j f) -> p j f", j=GT)
        nc.sync.dma_start(ch3[:, :, TD:TD + ED], e_d[grp])
        tq = gpool.tile([128, GT], F32, tag="tq")
        nc.sync.dma_start(tq[:], t_d[grp])
        dq = gpool.tile([128, GT], F32, tag="dq")
        nc.sync.dma_start(dq[:], d_d[grp])
        # batch all time-encodes of the group: keeps the ACT engine on one
        # function table (table reloads cost ~1.3us each)
        for col in range(GT):
            nc.scalar.activation(
                chunk[:, col * 128: col * 128 + TD], freqbc_t[:],
                mybir.ActivationFunctionType.Sin,
                bias=halfpi_t[:], scale=tq[:, col:col + 1])

        for wi in range(G):
            w = grp * G + wi
            hT = wpool.tile([128, 128], F32, tag="hT")
            nc.sync.dma_start(hT[:], hT_d[:, w * 128:(w + 1) * 128])
            hTb = wpool.tile([128, 128], BF, tag="hTb")
            nc.vector.tensor_copy(hTb[:], hT[:])
            # per-window two-zone gather with static quantized counts
            idx_t = wpool.tile([128, 2 * NZS // 16], I16, tag="idx")
            nc.sync.dma_start(idx_t[:], idx_d[w])
            gat = wpool.tile([128, TW * 128], BF, tag="gat")
            # zero the slot on its first 3 uses (one per pool buf) so skipped
            # padding slots never expose non-finite stale SBUF data
            ginit = war_state.setdefault("gat_init", [0])
            if ginit[0] < 3:
                nc.vector.memset(gat[:], 0.0)
                ginit[0] += 1
            g3 = gat[:].rearrange("p (c f) -> p c f", f=128)
            ra, rb = int(cq[w][0]), int(cq[w][1])
            nc.gpsimd.dma_gather(g3[:, 0:NZT, :], src_tab[:],
                                 idx_t[:, 0:NZS // 16], num_idxs=NZS,
                                 num_idxs_reg=ra, elem_size=128,
                                 single_packet=False)
            nc.gpsimd.dma_gather(g3[:, NZT:TW, :], src_tab[HALF:, :],
                                 idx_t[:, NZS // 16:], num_idxs=NZS,
                                 num_idxs_reg=rb, elem_size=128,
                                 single_packet=False)
            pseg = ps_seg.tile([128, 384], F32)
            for j in range(TW):
                col = wi * TW + j
                oh = wpool.tile([128, 128], BF, tag="oh")
                nc.vector.tensor_scalar(oh[:], iota_t[:], dq[:, col:col + 1], None,
                                        op0=mybir.AluOpType.is_equal)
                # One accumulation group per PSUM bank. start/stop flags mark
                # the zero region spanned by the matmul's PARTITIONS, so the
                # group must be opened and closed by full-128-partition
                # matmuls (the M=1 count matmul goes in the middle).
                mm_src = nc.tensor.matmul(pseg[:, 0:128],
                                          gat[:, j * 128:(j + 1) * 128], oh[:],
                                          start=(j == 0), stop=False)
                if j == 0:
                    _dep_on_prev_readers(mm_src, war_state, "pseg")
                nc.tensor.matmul(pseg[0:1, 256:384], ones_col[:], oh[:],
                                 start=False, stop=False)
                mm_chk = nc.tensor.matmul(pseg[:, 128:256],
                                          chunk[:, col * 128:(col + 1) * 128], oh[:],
                                          start=False, stop=(j == TW - 1))
                if j == TW - 1:
                    seg_closer = mm_chk
            # window epilogue: inv-count + indicator, broadcast via K=1 matmul
            bc_in = wpool.tile([1, 256], F32, tag="bc_in")
            cm = wpool.tile([1, 128], F32, tag="cm")
            i_cm = nc.vector.tensor_scalar_max(cm[:], pseg[0:1, 256:384], 1.0)
            add_dep_helper(i_cm.ins, seg_closer.ins, sync=True,
                           reason="cm reads pseg bank after group close")
            nc.vector.reciprocal(bc_in[:, 0:128], cm[:])
            i_ind = nc.vector.tensor_tensor(bc_in[:, 128:256], pseg[0:1, 256:384],
                                            bc_in[:, 0:128], op=mybir.AluOpType.mult)
            bcp = ps_bc.tile([128, 256], F32)
            i_bcp = nc.tensor.matmul(bcp[:], ones_row[:], bc_in[:],
                                     start=True, stop=True)
            _dep_on_prev_readers(i_bcp, war_state, "bc")
            bcs = wpool.tile([128, 256], F32, tag="bcs")
            i_bcs = nc.vector.tensor_copy(bcs[:], bcp[:])
            # mean chunks (transposed layout [m, n]); explicit deps on the
            # bank-group-closing matmul (readers of other regions)
            x0 = wpool.tile([128, 128], BF, tag="x0")
            i_x0 = nc.vector.tensor_tensor(x0[:], pseg[:, 0:128], bcs[:, 0:128],
                                           op=mybir.AluOpType.mult)
            add_dep_helper(i_x0.ins, seg_closer.ins, sync=True,
                           reason="x0 reads pseg bank after group close")
            x1 = wpool.tile([128, 128], BF, tag="x1")
            nc.vector.tensor_tensor(x1[:], hT[:], bcs[:, 128:256],
                                    op=mybir.AluOpType.mult)
            x2 = wpool.tile([128, 128], BF, tag="x2")
            i_x2 = nc.vector.tensor_tensor(x2[:], pseg[:, 128:256], bcs[:, 0:128],
                                           op=mybir.AluOpType.mult)
            add_dep_helper(i_x2.ins, seg_closer.ins, sync=True,
                           reason="x2 reads pseg bank after group close")
            xs = (x0, x1, x2)
            # GRU matmuls: pA regions j = gi_j (+ gh_j for j<2); pB = gh_2
            pA = ps_a.tile([128, 384], F32)
            pB = ps_b.tile([128, 128], F32)
            for jg in range(3):
                for cmi in range(3):
                    mm_a = nc.tensor.matmul(
                        pA[:, jg * 128:(jg + 1) * 128],
                        wih_t[:, (cmi * 3 + jg) * 128:(cmi * 3 + jg + 1) * 128],
                        xs[cmi][:], start=(jg == 0 and cmi == 0),
                        stop=(jg == 2 and cmi == 2))
                    if jg == 0 and cmi == 0:
                        _dep_on_prev_readers(mm_a, war_state, "pA")
                    if jg == 2 and cmi == 2:
                        a_closer = mm_a
                if jg < 2:
                    nc.tensor.matmul(
                        pA[:, jg * 128:(jg + 1) * 128],
                        whh_t[:, jg * 128:(jg + 1) * 128], hTb[:],
                        start=False, stop=False)
            i_pb = nc.tensor.matmul(pB[:], whh_t[:, 256:384], hTb[:],
                                    start=True, stop=True)
            _dep_on_prev_readers(i_pb, war_state, "pB")
            # gates
            r = wpool.tile([128, 128], F32, tag="r")
            i_r = nc.scalar.activation(r[:], pA[:, 0:128],
                                       mybir.ActivationFunctionType.Sigmoid,
                                       bias=bias4[:, 0:1])
            add_dep_helper(i_r.ins, a_closer.ins, sync=True,
                           reason="r reads pA bank after group close")
            z = wpool.tile([128, 128], F32, tag="z")
            i_z = nc.scalar.activation(z[:], pA[:, 128:256],
                                       mybir.ActivationFunctionType.Sigmoid,
                                       bias=bias4[:, 1:2])
            add_dep_helper(i_z.ins, a_closer.ins, sync=True,
                           reason="z reads pA bank after group close")
            v1 = wpool.tile([128, 128], F32, tag="v1")
            i_v1 = nc.vector.tensor_scalar_add(v1[:], pB[:], bias4[:, 3:4])
            v2 = wpool.tile([128, 128], F32, tag="v2")
            nc.vector.tensor_tensor(v2[:], v1[:], r[:], op=mybir.AluOpType.mult)
            v3 = wpool.tile([128, 128], F32, tag="v3")
            i_v3 = nc.vector.tensor_tensor(v3[:], v2[:], pA[:, 256:384],
                                           op=mybir.AluOpType.add)
            # tanh(x) = 2*sigmoid(2x) - 1: keeps the ACT engine on the
            # Sigmoid table (bias4 col 2 holds 2*b_ihn, scale doubles v3)
            nsig = wpool.tile([128, 128], F32, tag="nsig")
            nc.scalar.activation(nsig[:], v3[:],
                                 mybir.ActivationFunctionType.Sigmoid,
                                 bias=bias4[:, 2:3], scale=2.0)
            ngate = wpool.tile([128, 128], F32, tag="n")
            nc.vector.tensor_scalar(ngate[:], nsig[:], 2.0, -1.0,
                                    op0=mybir.AluOpType.mult,
                                    op1=mybir.AluOpType.add)
            d1 = wpool.tile([128, 128], F32, tag="d1")
            nc.vector.tensor_tensor(d1[:], hT[:], ngate[:],
                                    op=mybir.AluOpType.subtract)
            zd = wpool.tile([128, 128], F32, tag="zd")
            nc.vector.tensor_tensor(zd[:], z[:], d1[:], op=mybir.AluOpType.mult)
            o = wpool.tile([128, 128], F32, tag="o")
            nc.vector.tensor_tensor(o[:], ngate[:], zd[:], op=mybir.AluOpType.add)
            nc.sync.dma_start(out_d[:, w * 128:(w + 1) * 128], o[:])
            # record psum readers of this window for WAR enforcement
            war_state.setdefault("pseg", []).append([i_x0, i_x2, i_cm, i_ind])
            war_state.setdefault("bc", []).append([i_bcs])
            war_state.setdefault("pA", []).append([i_r, i_z, i_v3])
            war_state.setdefault("pB", []).append([i_v1])


def _build(cq_g, cq_r):
    nc = bacc.Bacc("TRN2", target_bir_lowering=False, debug=False)

    si_full = nc.dram_tensor("si_full", [NPAD, S], BF, kind="ExternalInput")
    sir_full = nc.dram_tensor("sir_full", [NPAD, S], BF, kind="ExternalInput")
    hTg = nc.dram_tensor("hTg", [S, NPC], F32, kind="ExternalInput")
    hTr = nc.dram_tensor("hTr", [S, NPC], F32, kind="ExternalInput")
    io = {}
    for nm in ("g", "r"):
        io["idx_" + nm] = nc.dram_tensor("idx_" + nm, [NW, 128, 2 * NZS // 16],
                                         I16, kind="ExternalInput")
        io["e_" + nm] = nc.dram_tensor("e_" + nm, [NG, 128, GT, ED], BF,
                                       kind="ExternalInput")
        io["t_" + nm] = nc.dram_tensor("t_" + nm, [NG, 128, GT], F32,
                                       kind="ExternalInput")
        io["d_" + nm] = nc.dram_tensor("d_" + nm, [NG, 128, GT], F32,
                                       kind="ExternalInput")
        io["outT_" + nm] = nc.dram_tensor("outT_" + nm, [S, NPC], F32,
                                          kind="ExternalOutput")
    wih = nc.dram_tensor("wih_t", [128, 9 * 128], BF, kind="ExternalInput")
    whh = nc.dram_tensor("whh_t", [S, 384], BF, kind="ExternalInput")
    bias4 = nc.dram_tensor("bias4", [128, 4], F32, kind="ExternalInput")
    iota_d = nc.dram_tensor("iota", [128, 128], F32, kind="ExternalInput")
    freqbc_d = nc.dram_tensor("freqbc", [128, TD], F32, kind="ExternalInput")

    with tile.TileContext(nc) as tc:
        with (
            tc.tile_pool(name="cst", bufs=1) as cpool,
            tc.tile_pool(name="grp", bufs=2) as gpool,
            tc.tile_pool(name="win", bufs=3) as wpool,
            tc.tile_pool(name="ps_seg", bufs=2, space="PSUM") as ps_seg,
            tc.tile_pool(name="ps_bc", bufs=2, space="PSUM") as ps_bc,
            tc.tile_pool(name="ps_a", bufs=2, space="PSUM") as ps_a,
            tc.tile_pool(name="ps_b", bufs=2, space="PSUM") as ps_b,
        ):
            wih_t = cpool.tile([128, 9 * 128], BF)
            nc.sync.dma_start(wih_t[:], wih[:])
            whh_t = cpool.tile([S, 384], BF)
            nc.sync.dma_start(whh_t[:], whh[:])
            bias_t = cpool.tile([128, 4], F32)
            nc.sync.dma_start(bias_t[:], bias4[:])
            iota_t = cpool.tile([128, 128], F32)
            nc.sync.dma_start(iota_t[:], iota_d[:])
            freqbc_t = cpool.tile([128, TD], F32)
            nc.sync.dma_start(freqbc_t[:], freqbc_d[:])
            halfpi_t = cpool.tile([128, 1], F32)
            nc.vector.memset(halfpi_t[:], float(np.pi / 2))
            ones_col = cpool.tile([128, 1], BF)
            nc.vector.memset(ones_col[:], 1.0)
            ones_row = cpool.tile([1, 128], F32)
            nc.vector.memset(ones_row[:], 1.0)
            pools = (gpool, wpool, ps_seg, ps_bc, ps_a, ps_b)
            cst = (wih_t, whh_t, bias_t, iota_t, freqbc_t, halfpi_t,
                   ones_col, ones_row)
            war_state = {}
            _emit_graph(nc, tc, pools, cst,
                        (si_full, hTg, io["idx_g"], io["e_g"],
                         io["t_g"], io["d_g"], io["outT_g"]),
                        war_state, cq_g)
            _emit_graph(nc, tc, pools, cst,
                        (sir_full, hTr, io["idx_r"], io["e_r"],
                         io["t_r"], io["d_r"], io["outT_r"]),
                        war_state, cq_r)
    nc.compile()
    return nc


def _wrap_idx(vals, nvalid):
    """int16 index list (0-padded to nvalid, -1 beyond) -> dma_gather SBUF
    layout [128, NZS//16]: idx i at [i%16 + 16*replica, i//16], replicated
    for the 8 Q7 cores."""
    full = np.full(NZS, -1, np.int16)
    full[:nvalid] = 0
    full[:len(vals)] = vals.astype(np.int16)
    blk = full.reshape(NZS // 16, 16).T  # [16, 48]
    return np.tile(blk, (8, 1))


QUANT = 64  # descriptor-count quantum (static counts shared across cores)


def _prep_graph(src, dst, t, e):
    """Sort by dst, shard by dst range across cores, split each window's
    edges into two src zones (int16 gather limit), pack into tile slots.
    Returns per-core arrays plus static quantized per-window counts."""
    order = np.argsort(dst, kind='stable')
    ds = dst[order].astype(np.int64)
    ss = src[order].astype(np.int64)
    ts = t[order].astype(np.float32)
    es = e[order].astype(np.float32)
    wb = np.searchsorted(ds, np.arange(0, NPAD + 1, 128))
    zidx = {}   # (c, w, zone) -> index values
    slot_a = np.zeros((NCORES, NG, 128, GT), np.int64)  # for emulation
    d_a = np.full((NCORES, NG, 128, GT), 200.0, np.float32)
    t_a = np.zeros((NCORES, NG, 128, GT), np.float32)
    e_a = np.zeros((NCORES, NG, 128, GT, ED), np.float32)
    for c in range(NCORES):
        for w in range(NW):
            gw = c * NW + w
            lo, hi = int(wb[gw]), int(wb[gw + 1])
            grp, wi = w // G, w % G
            sseg = ss[lo:hi]
            za = np.nonzero(sseg < HALF)[0]
            zb = np.nonzero(sseg >= HALF)[0]
            assert len(za) <= NZS and len(zb) <= NZS, \
                f"zone overflow: {len(za)}/{len(zb)}"
            zidx[(c, w, 0)] = sseg[za]
            zidx[(c, w, 1)] = sseg[zb] - HALF
            for zi, zz in ((0, za), (1, zb)):
                n = len(zz)
                if n == 0:
                    continue
                k = np.arange(n)
                p = k % 128
                cols = wi * TW + zi * NZT + k // 128
                sel = lo + zz
                slot_a[c, grp, p, cols] = ss[sel]
                d_a[c, grp, p, cols] = (ds[sel] - gw * 128).astype(np.float32)
                t_a[c, grp, p, cols] = ts[sel]
                e_a[c, grp, p, cols, :] = es[sel]
    # static counts: max over cores, quantized up
    cq = []
    for w in range(NW):
        row = []
        for zi in range(2):
            m = max(len(zidx[(c, w, zi)]) for c in range(NCORES))
            row.append(min(NZS, max(QUANT, -(-m // QUANT) * QUANT)))
        cq.append(tuple(row))
    idx_a = np.zeros((NCORES, NW, 128, 2 * NZS // 16), np.int16)
    for c in range(NCORES):
        for w in range(NW):
            idx_a[c, w, :, :NZS // 16] = _wrap_idx(zidx[(c, w, 0)], cq[w][0])
            idx_a[c, w, :, NZS // 16:] = _wrap_idx(zidx[(c, w, 1)], cq[w][1])
    return idx_a, cq, slot_a, d_a, t_a, e_a


last_results = None


def kernel(si, sj, si_r, sj_r, t, t_r, e, e_r,
           w_ih, w_hh, b_ih, b_hh,
           src_g, dst_g, src_gr, dst_gr):
    global last_results
    si = np.asarray(si, np.float32)
    sj = np.asarray(sj, np.float32)
    si_r = np.asarray(si_r, np.float32)
    sj_r = np.asarray(sj_r, np.float32)

    def padrows(a):
        out = np.zeros((NPAD, S), np.float32)
        out[:a.shape[0]] = a
        return out

    si_p = padrows(si)
    sir_p = padrows(si_r)
    sj_p = padrows(sj)
    sjr_p = padrows(sj_r)

    ig, cg, _sg, dg, tg, eg = _prep_graph(np.asarray(src_g), np.asarray(dst_g),
                                          np.asarray(t), np.asarray(e))
    ir_, cr, _sr, dr, tr, er = _prep_graph(np.asarray(src_gr), np.asarray(dst_gr),
                                           np.asarray(t_r), np.asarray(e_r))

    key = (tuple(cg), tuple(cr))
    if _CACHE.get("key") != key:
        _CACHE["nc"] = _build(cg, cr)
        _CACHE["key"] = key
    nc = _CACHE["nc"]

    wih_t = np.ascontiguousarray(np.asarray(w_ih, np.float32).T)  # [384 m, 384 g]
    wih_host = np.zeros((128, 9 * 128), np.float32)
    for cm in range(3):
        for jg in range(3):
            wih_host[:, (cm * 3 + jg) * 128:(cm * 3 + jg + 1) * 128] = \
                wih_t[cm * 128:(cm + 1) * 128, jg * 128:(jg + 1) * 128]
    whh_host = np.ascontiguousarray(np.asarray(w_hh, np.float32).T)  # [128, 384]
    b_ih = np.asarray(b_ih, np.float32)
    b_hh = np.asarray(b_hh, np.float32)
    bias_host = np.stack([
        (b_ih + b_hh)[0:128], (b_ih + b_hh)[128:256],
        2.0 * b_ih[256:384], b_hh[256:384]], axis=1).astype(np.float32)
    iota_host = np.tile(np.arange(128, dtype=np.float32), (128, 1))
    freq = (1.0 / 10 ** np.linspace(0, 9, TD)).astype(np.float32)
    freqbc_host = np.tile(freq, (128, 1))

    si_b = si_p.astype(ml_dtypes.bfloat16)
    sir_b = sir_p.astype(ml_dtypes.bfloat16)
    eg_b = eg.astype(ml_dtypes.bfloat16)
    er_b = er.astype(ml_dtypes.bfloat16)
    wih_b = wih_host.astype(ml_dtypes.bfloat16)
    whh_b = whh_host.astype(ml_dtypes.bfloat16)
    in_maps = []
    for c in range(NCORES):
        sl = slice(c * NPC, (c + 1) * NPC)
        in_maps.append({
            "si_full": si_b, "sir_full": sir_b,
            "hTg": np.ascontiguousarray(sj_p[sl].T),
            "hTr": np.ascontiguousarray(sjr_p[sl].T),
            "idx_g": ig[c], "e_g": eg_b[c], "t_g": tg[c], "d_g": dg[c],
            "idx_r": ir_[c], "e_r": er_b[c], "t_r": tr[c], "d_r": dr[c],
            "wih_t": wih_b, "whh_t": whh_b, "bias4": bias_host,
            "iota": iota_host, "freqbc": freqbc_host,
        })

    res = bass_utils.run_bass_kernel_spmd(nc, in_maps, core_ids=list(range(NCORES)))
    last_results = res

    sj_out = np.concatenate([res.results[c]["outT_g"].T for c in range(NCORES)],
                            axis=0)[:NN]
    si_out = np.concatenate([res.results[c]["outT_r"].T for c in range(NCORES)],
                            axis=0)[:NN]
    return (si_out, sj_out)
